# revision 1
# baseline (speedup 1.0000x reference)
"""Neural CDE (RK4, 10 steps) Trainium2 Bass/Tile kernel.

Data-parallel over batch: B=1024 split as 128 per core across 8 NeuronCores.
Weights replicated; no collectives.

Per-core math (BS=128 on SBUF partitions):
  z0 = a[:,0] @ W_init + b_init
  per RK4 stage:  dX is one of 21 precomputed vectors (spline derivative at
  the stage's time, which only depends on coeffs, not z):
      hT   = tanh(W1.T-contract zT + b1)           (PE + ACT, h on partitions)
      F    = hT.T @ W2  in 1024-wide chunks        (PE, bf16, f32 PSUM accum)
      k    = segmented-reduce_c(F * rep(dX)) + dX @ b2r.T   (ACT copy, DVE
             mul + reduce, small PE matmul for the b2 term)
  RK4 combine in f32 on DVE. Output out[t] = z_t @ W_out + b_out per step.
"""

import sys
import numpy as np

for _p in ("/opt/trn_rl_repo",):
    if _p not in sys.path:
        sys.path.insert(0, _p)

import ml_dtypes
from contextlib import ExitStack

import concourse.bass as bass
import concourse.bacc as bacc
import concourse.mybir as mybir
import concourse.tile as tile
from concourse.masks import make_identity
from concourse.bass_utils import run_bass_kernel_spmd

B, T, C, H = 1024, 11, 64, 256
NCORES = 8
BS = B // NCORES          # 128
HC = H * C                # 16384
CHUNK = 1024              # F free-dim chunk = 2 matmul windows of 512
NCHUNK = HC // CHUNK      # 16
NW = CHUNK // 512         # windows per chunk

f32 = np.float32
bf16 = ml_dtypes.bfloat16
FP32 = mybir.dt.float32
BF16 = mybir.dt.bfloat16
AO = mybir.AluOpType
AF = mybir.ActivationFunctionType
AX = mybir.AxisListType


def _stage_consts(t_span: np.ndarray):
    """Host-side f32 scalar constants mimicking the reference's fp32 ops."""
    t = np.asarray(t_span, dtype=f32)
    cs = []
    for i in range(T - 1):
        t0 = t[i]
        dt = f32(t[i + 1] - t0)
        tm = f32(t0 + f32(f32(0.5) * dt))
        idx_m = int(np.clip(np.searchsorted(t, tm, side="right") - 1, 0, T - 2))
        fm = f32(tm - t[idx_m])
        cs.append((float(dt), idx_m, float(fm)))
    # final-stage frac for step T-2 (t lands on t_span[-1], idx clamps to T-2)
    fr_last = f32(t[T - 1] - t[T - 2])
    return cs, float(fr_last)


def _build_program(t_span: np.ndarray):
    cs, fr_last = _stage_consts(t_span)

    nc = bacc.Bacc("TRN2", target_bir_lowering=False, debug=False,
                   enable_asserts=False, num_devices=NCORES)

    coeffs_d = nc.dram_tensor("coeffs", [BS, T - 1, 4 * C], FP32, kind="ExternalInput").ap()
    w1_d = nc.dram_tensor("w1", [H, H], BF16, kind="ExternalInput").ap()
    w2_d = nc.dram_tensor("w2", [H, HC], BF16, kind="ExternalInput").ap()
    b1_d = nc.dram_tensor("b1", [H], FP32, kind="ExternalInput").ap()
    b2rt_d = nc.dram_tensor("b2rt", [C, H], BF16, kind="ExternalInput").ap()
    winit_d = nc.dram_tensor("winit", [C, H], BF16, kind="ExternalInput").ap()
    wout_d = nc.dram_tensor("wout", [H, C], FP32, kind="ExternalInput").ap()
    binit_d = nc.dram_tensor("binit", [1, H], FP32, kind="ExternalInput").ap()
    bout_d = nc.dram_tensor("bout", [1, C], FP32, kind="ExternalInput").ap()
    out_d = nc.dram_tensor("out", [BS, T * C], FP32, kind="ExternalOutput").ap()

    with tile.TileContext(nc) as tc, ExitStack() as ctx:
        const = ctx.enter_context(tc.tile_pool(name="const", bufs=1))
        spool = ctx.enter_context(tc.tile_pool(name="stage", bufs=2))
        zpool = ctx.enter_context(tc.tile_pool(name="z", bufs=2))
        kbpool = ctx.enter_context(tc.tile_pool(name="kb", bufs=5))
        fpool = ctx.enter_context(tc.tile_pool(name="fsb", bufs=6))
        gpool = ctx.enter_context(tc.tile_pool(name="gsb", bufs=4))
        pp = ctx.enter_context(tc.tile_pool(name="psmm", bufs=4, space="PSUM"))
        fp = ctx.enter_context(tc.tile_pool(name="psfp", bufs=2, space="PSUM"))

        # ---- resident tensors -------------------------------------------
        coeffs_sb = const.tile([BS, (T - 1) * 4 * C], FP32, tag="coeffs")
        w1_sb = const.tile([128, 2 * H], BF16, tag="w1")
        w2_sb = const.tile([128, 2 * HC], BF16, tag="w2")
        b1_sb = const.tile([128, 2], FP32, tag="b1")
        b2rt_sb = const.tile([C, H], BF16, tag="b2rt")
        winit_sb = const.tile([C, H], BF16, tag="winit")
        wout_sb = const.tile([128, 2 * C], FP32, tag="wout")
        binit_sb = const.tile([1, H], FP32, tag="binit")
        bout_sb = const.tile([1, C], FP32, tag="bout")
        ones1_sb = const.tile([1, 128], FP32, tag="ones1")
        ident = const.tile([128, 128], FP32, tag="ident")
        binit_rep = const.tile([128, H], FP32, tag="binit_rep")
        bout_rep = const.tile([128, C], FP32, tag="bout_rep")
        dxm_sb = const.tile([128, 11 * C], FP32, tag="dxm")      # 10 mids + last-end
        dxT_sb = const.tile([C, 21 * 128], BF16, tag="dxT")
        dxrep_sb = const.tile([128, 21 * CHUNK], BF16, tag="dxrep")
        out_sb = const.tile([BS, T * C], FP32, tag="out_sb")

        nc.sync.dma_start(out=coeffs_sb[:], in_=coeffs_d.rearrange("p i j -> p (i j)"))
        nc.sync.dma_start(out=w1_sb.rearrange("p (k h) -> p k h", k=2),
                          in_=w1_d.rearrange("(k p) h -> p k h", p=128))
        nc.sync.dma_start(out=w2_sb.rearrange("p (k m) -> p k m", k=2),
                          in_=w2_d.rearrange("(k p) m -> p k m", p=128))
        nc.sync.dma_start(out=b1_sb[:], in_=b1_d.rearrange("(k p) -> p k", p=128))
        nc.sync.dma_start(out=b2rt_sb[:], in_=b2rt_d)
        nc.sync.dma_start(out=winit_sb[:], in_=winit_d)
        nc.sync.dma_start(out=wout_sb.rearrange("p (k c) -> p k c", k=2),
                          in_=wout_d.rearrange("(k p) c -> p k c", p=128))
        nc.sync.dma_start(out=binit_sb[:], in_=binit_d)
        nc.sync.dma_start(out=bout_sb[:], in_=bout_d)

        nc.vector.memset(ones1_sb[:], 1.0)
        make_identity(nc, ident[:])

        def cview(i, part):
            """f32 view of coeff column `part` (0=a,1=b,2=2c,3=3d) of interval i."""
            off = i * 4 * C + part * C
            return coeffs_sb[:, off:off + C]

        def dx_f32(s):
            if s < 10:
                return cview(s, 1)
            return dxm_sb[:, (s - 10) * C:(s - 9) * C]

        # ---- dX mid/end vectors (f32) -----------------------------------
        tmp_pool = ctx.enter_context(tc.tile_pool(name="tmp64", bufs=2))
        for i in range(T - 1):
            dt_i, im, fm = cs[i]
            tmp = tmp_pool.tile([128, C], FP32, tag="t64")
            nc.vector.scalar_tensor_tensor(
                out=tmp[:], in0=cview(im, 3), scalar=float(fm), in1=cview(im, 2),
                op0=AO.mult, op1=AO.add)
            nc.vector.scalar_tensor_tensor(
                out=dxm_sb[:, i * C:(i + 1) * C], in0=tmp[:], scalar=float(fm),
                in1=cview(im, 1), op0=AO.mult, op1=AO.add)
        # end-of-grid vector for the very last stage (s == 20)
        tmp = tmp_pool.tile([128, C], FP32, tag="t64")
        nc.vector.scalar_tensor_tensor(
            out=tmp[:], in0=cview(T - 2, 3), scalar=float(fr_last), in1=cview(T - 2, 2),
            op0=AO.mult, op1=AO.add)
        nc.vector.scalar_tensor_tensor(
            out=dxm_sb[:, 10 * C:11 * C], in0=tmp[:], scalar=float(fr_last),
            in1=cview(T - 2, 1), op0=AO.mult, op1=AO.add)

        # ---- dX transposes (bf16) + repeated copies (bf16) ---------------
        for s in range(21):
            src = dx_f32(s)
            ps = pp.tile([128, H], FP32, tag="mm")
            nc.tensor.transpose(ps[0:C, 0:128], src, ident[:])
            nc.scalar.copy(dxT_sb[:, s * 128:(s + 1) * 128], ps[0:C, 0:128])
            for r in range(CHUNK // C):
                nc.scalar.copy(dxrep_sb[:, s * CHUNK + r * C: s * CHUNK + (r + 1) * C], src)

        # ---- replicated biases -------------------------------------------
        ps = pp.tile([128, H], FP32, tag="mm")
        nc.tensor.matmul(ps[:, 0:H], lhsT=ones1_sb[:], rhs=binit_sb[:], start=True, stop=True)
        nc.scalar.copy(binit_rep[:], ps[:, 0:H])
        ps = pp.tile([128, H], FP32, tag="mm")
        nc.tensor.matmul(ps[:, 0:C], lhsT=ones1_sb[:], rhs=bout_sb[:], start=True, stop=True)
        nc.scalar.copy(bout_rep[:], ps[:, 0:C])

        # ---- z0 ----------------------------------------------------------
        ps = pp.tile([128, H], FP32, tag="mm")
        nc.tensor.transpose(ps[0:C, 0:128], cview(0, 0), ident[:])
        x0T_sb = spool.tile([C, 128], BF16, tag="x0T")
        nc.scalar.copy(x0T_sb[:], ps[0:C, 0:128])
        ps = pp.tile([128, H], FP32, tag="mm")
        nc.tensor.matmul(ps[:, 0:H], lhsT=x0T_sb[:], rhs=winit_sb[:], start=True, stop=True)
        z = zpool.tile([BS, H], FP32, tag="z")
        nc.vector.tensor_tensor(out=z[:], in0=ps[:, 0:H], in1=binit_rep[:], op=AO.add)

        # ---- one RK4 stage ----------------------------------------------
        # Returns (ksum, bc_ps, zb, pre): ksum = segmented-reduced F*dX;
        # bc_ps = dX @ b2r.T (PSUM); zb = zbase + alpha*bc (hoisted off the
        # critical tail); pre = pre_add + bc (for the RK4 combine).
        def gstage(zin, s, alpha=None, zbase=None, pre_add=None, emit_out_t=None):
            # transpose zin -> zT (two separate PSUM tiles = different banks, so
            # the copy of half 0 overlaps the transpose of half 1)
            zt_psA = pp.tile([128, 128], FP32, tag="mm")
            zt_psB = pp.tile([128, 128], FP32, tag="mm")
            nc.tensor.transpose(zt_psA[:], zin[:, 0:128], ident[:])
            nc.tensor.transpose(zt_psB[:], zin[:, 128:256], ident[:])
            zTb = spool.tile([128, H], BF16, tag="zTb")
            nc.scalar.copy(zTb[:, 0:128], zt_psA[:])
            nc.scalar.copy(zTb[:, 128:256], zt_psB[:])

            # b2-term: bc = dX @ b2r.T   (PSUM, lives through the stage)
            bc_ps = pp.tile([128, H], FP32, tag="mm")
            nc.tensor.matmul(bc_ps[:], lhsT=dxT_sb[:, s * 128:(s + 1) * 128],
                             rhs=b2rt_sb[:], start=True, stop=True)
            zb = None
            if alpha is not None:
                zb = zpool.tile([BS, H], FP32, tag="zb")
                nc.vector.scalar_tensor_tensor(out=zb[:], in0=bc_ps[:], scalar=float(alpha),
                                               in1=zbase[:], op0=AO.mult, op1=AO.add)
            pre = None
            if pre_add is not None:
                pre = kbpool.tile([BS, H], FP32, tag="pre")
                nc.vector.tensor_tensor(out=pre[:], in0=pre_add[:], in1=bc_ps[:], op=AO.add)

            if emit_out_t is not None:
                t_idx = emit_out_t
                zTf = spool.tile([128, H], FP32, tag="zTf")
                nc.scalar.copy(zTf[:, 0:128], zt_psA[:])
                nc.scalar.copy(zTf[:, 128:256], zt_psB[:])
                ot_ps = pp.tile([128, H], FP32, tag="mm")
                for kc in range(2):
                    nc.tensor.matmul(ot_ps[:, 0:C], lhsT=zTf[:, kc * 128:(kc + 1) * 128],
                                     rhs=wout_sb[:, kc * C:(kc + 1) * C],
                                     start=(kc == 0), stop=(kc == 1))
                nc.vector.tensor_tensor(out=out_sb[:, t_idx * C:(t_idx + 1) * C],
                                        in0=ot_ps[:, 0:C], in1=bout_rep[:], op=AO.add)

            # hT = tanh(W1.T zT + b1)
            ht_ps = pp.tile([128, H], FP32, tag="mm")
            for hck in range(2):
                for kc in range(2):
                    nc.tensor.matmul(
                        ht_ps[:, hck * 128:(hck + 1) * 128],
                        lhsT=w1_sb[:, kc * H + hck * 128: kc * H + (hck + 1) * 128],
                        rhs=zTb[:, kc * 128:(kc + 1) * 128],
                        start=(kc == 0), stop=(kc == 1))
            hT0 = spool.tile([128, 128], BF16, tag="hT0")
            hT1 = spool.tile([128, 128], BF16, tag="hT1")
            for hck, ht_t in enumerate((hT0, hT1)):
                nc.scalar.activation(ht_t[:],
                                     ht_ps[:, hck * 128:(hck + 1) * 128],
                                     AF.Tanh, bias=b1_sb[:, hck:hck + 1], scale=1.0)
            hT_half = (hT0, hT1)

            # F chunks -> scaled -> segment-reduced
            ksum = kbpool.tile([BS, H], FP32, tag="ksum")
            # first two chunks are half-width so the ACT/DVE pipeline primes sooner
            chunks = [(0, 512), (512, 512)] + [(1024 * (j + 1), 1024) for j in range(15)]
            for off, cw in chunks:
                fps = fp.tile([128, cw], FP32, tag="fp")
                for kc in range(2):
                    for w in range(cw // 512):
                        col = kc * HC + off + w * 512
                        nc.tensor.matmul(fps[:, w * 512:(w + 1) * 512],
                                         lhsT=hT_half[kc][:],
                                         rhs=w2_sb[:, col:col + 512],
                                         start=(kc == 0), stop=(kc == 1),
                                         skip_group_check=True)
                fsb = fpool.tile([128, cw], BF16, tag="fsb")
                nc.scalar.copy(fsb[:], fps[:])
                gsb = gpool.tile([128, cw], BF16, tag="gsb")
                nc.vector.tensor_tensor(out=gsb[:], in0=fsb[:],
                                        in1=dxrep_sb[:, s * CHUNK:s * CHUNK + cw],
                                        op=AO.mult)
                nc.vector.tensor_reduce(
                    out=ksum[:, off // C:(off + cw) // C],
                    in_=gsb.rearrange("p (s c) -> p s c", c=C),
                    axis=AX.X, op=AO.add)
            return ksum, bc_ps, zb, pre

        # ---- RK4 time loop ----------------------------------------------
        for i in range(T - 1):
            dt_i, im, fm = cs[i]
            hdt = float(f32(f32(0.5) * f32(dt_i)))
            dt6 = float(f32(f32(dt_i) / f32(6.0)))
            s_m = 10 + i
            s_e = (i + 1) if i < T - 2 else 20

            def kfull(ksum, bc_ps):
                kb = kbpool.tile([BS, H], FP32, tag="kb")
                nc.vector.tensor_tensor(out=kb[:], in0=ksum[:], in1=bc_ps[:], op=AO.add)
                return kb

            k1s, bc1, zb1, _ = gstage(z, i, alpha=hdt, zbase=z, emit_out_t=i)
            zs = zpool.tile([BS, H], FP32, tag="zs")
            for hh in (slice(0, 128), slice(128, 256)):
                nc.vector.scalar_tensor_tensor(out=zs[:, hh], in0=k1s[:, hh], scalar=hdt,
                                               in1=zb1[:, hh], op0=AO.mult, op1=AO.add)
            kb1 = kfull(k1s, bc1)

            k2s, bc2, zb2, _ = gstage(zs, s_m, alpha=hdt, zbase=z)
            zs = zpool.tile([BS, H], FP32, tag="zs")
            for hh in (slice(0, 128), slice(128, 256)):
                nc.vector.scalar_tensor_tensor(out=zs[:, hh], in0=k2s[:, hh], scalar=hdt,
                                               in1=zb2[:, hh], op0=AO.mult, op1=AO.add)
            kb2 = kfull(k2s, bc2)

            k3s, bc3, zb3, _ = gstage(zs, s_m, alpha=float(dt_i), zbase=z)
            zs = zpool.tile([BS, H], FP32, tag="zs")
            for hh in (slice(0, 128), slice(128, 256)):
                nc.vector.scalar_tensor_tensor(out=zs[:, hh], in0=k3s[:, hh], scalar=float(dt_i),
                                               in1=zb3[:, hh], op0=AO.mult, op1=AO.add)
            kb3 = kfull(k3s, bc3)

            # partial RK4 combine (ready before k4's reduces finish)
            acc = kbpool.tile([BS, H], FP32, tag="acc")
            nc.vector.scalar_tensor_tensor(out=acc[:], in0=kb2[:], scalar=2.0, in1=kb1[:],
                                           op0=AO.mult, op1=AO.add)
            acc2 = kbpool.tile([BS, H], FP32, tag="acc2")
            nc.vector.scalar_tensor_tensor(out=acc2[:], in0=kb3[:], scalar=2.0, in1=acc[:],
                                           op0=AO.mult, op1=AO.add)

            k4s, _, _, pre = gstage(zs, s_e, pre_add=acc2)
            acc3 = kbpool.tile([BS, H], FP32, tag="acc3")
            znew = zpool.tile([BS, H], FP32, tag="z")
            for hh in (slice(0, 128), slice(128, 256)):
                nc.vector.tensor_tensor(out=acc3[:, hh], in0=k4s[:, hh], in1=pre[:, hh], op=AO.add)
                nc.vector.scalar_tensor_tensor(out=znew[:, hh], in0=acc3[:, hh], scalar=dt6,
                                               in1=z[:, hh], op0=AO.mult, op1=AO.add)
            z = znew

        # ---- final out row (t = T-1) ------------------------------------
        zt_psA = pp.tile([128, 128], FP32, tag="mm")
        zt_psB = pp.tile([128, 128], FP32, tag="mm")
        nc.tensor.transpose(zt_psA[:], z[:, 0:128], ident[:])
        nc.tensor.transpose(zt_psB[:], z[:, 128:256], ident[:])
        zTf = spool.tile([128, H], FP32, tag="zTf")
        nc.scalar.copy(zTf[:, 0:128], zt_psA[:])
        nc.scalar.copy(zTf[:, 128:256], zt_psB[:])
        ot_ps = pp.tile([128, H], FP32, tag="mm")
        for kc in range(2):
            nc.tensor.matmul(ot_ps[:, 0:C], lhsT=zTf[:, kc * 128:(kc + 1) * 128],
                             rhs=wout_sb[:, kc * C:(kc + 1) * C],
                             start=(kc == 0), stop=(kc == 1))
        nc.vector.tensor_tensor(out=out_sb[:, (T - 1) * C:T * C],
                                in0=ot_ps[:, 0:C], in1=bout_rep[:], op=AO.add)

        nc.sync.dma_start(out=out_d, in_=out_sb[:])

    nc.compile()
    return nc


_CACHE = {}


def _get_program(t_span: np.ndarray):
    key = np.asarray(t_span, dtype=f32).tobytes()
    if key not in _CACHE:
        _CACHE[key] = _build_program(t_span)
    return _CACHE[key]


def _make_in_maps(inputs):
    coeffs = np.ascontiguousarray(inputs["coeffs"], dtype=f32)
    assert coeffs.shape == (B, T - 1, 4 * C)
    shared = {
        "w1": np.ascontiguousarray(inputs["W1"], dtype=f32).astype(bf16),
        "w2": np.ascontiguousarray(inputs["W2"], dtype=f32).astype(bf16),
        "b1": np.ascontiguousarray(inputs["b1"], dtype=f32),
        "b2rt": np.ascontiguousarray(
            np.asarray(inputs["b2"], dtype=f32).reshape(H, C).T).astype(bf16),
        "winit": np.ascontiguousarray(inputs["W_init"], dtype=f32).astype(bf16),
        "wout": np.ascontiguousarray(inputs["W_out"], dtype=f32),
        "binit": np.ascontiguousarray(inputs["b_init"], dtype=f32).reshape(1, H),
        "bout": np.ascontiguousarray(inputs["b_out"], dtype=f32).reshape(1, C),
    }
    in_maps = []
    for c in range(NCORES):
        m = dict(shared)
        m["coeffs"] = coeffs[c * BS:(c + 1) * BS]
        in_maps.append(m)
    return in_maps


def kernel(coeffs, t_span, W_init, b_init, W1, b1, W2, b2, W_out, b_out):
    nc = _get_program(t_span)
    in_maps = _make_in_maps(dict(coeffs=coeffs, W_init=W_init, b_init=b_init,
                                 W1=W1, b1=b1, W2=W2, b2=b2,
                                 W_out=W_out, b_out=b_out))
    res = run_bass_kernel_spmd(nc, in_maps, list(range(NCORES)))
    shards = [res.results[c]["out"].reshape(BS, T, C) for c in range(NCORES)]
    return np.ascontiguousarray(np.concatenate(shards, axis=0), dtype=f32)


if __name__ == "__main__":
    rng = np.random.default_rng(0)
    demo = dict(
        coeffs=(rng.standard_normal((B, T - 1, 4 * C)) * 0.5).astype(f32),
        t_span=(np.arange(T) * 0.05).astype(f32),
        W_init=(rng.standard_normal((C, H)) / 8).astype(f32),
        b_init=(rng.standard_normal((H,)) * 0.01).astype(f32),
        W1=(rng.standard_normal((H, H)) / 16).astype(f32),
        b1=(rng.standard_normal((H,)) * 0.01).astype(f32),
        W2=(rng.standard_normal((H, HC)) / 16).astype(f32),
        b2=(rng.standard_normal((HC,)) * 0.01).astype(f32),
        W_out=(rng.standard_normal((H, C)) / 16).astype(f32),
        b_out=np.zeros((C,), f32),
    )
    out = kernel(**demo)
    print("out", out.shape, out.dtype, float(np.abs(out).max()))



# revision 9
# speedup vs baseline: 1.1895x; 1.1895x over previous
"""Neural CDE (RK4, 10 steps) Trainium2 Bass/Tile kernel — fp8 rewrite.

Data-parallel over batch: B=1024 split as 128 per core across 8 NeuronCores.
Weights replicated; no collectives.

Key idea vs the bf16 baseline: fold the spline derivative dX into the
matmul's *stationary* operand.  For each channel c the per-stage vector
field contraction

    k[b,h] = sum_{j,c} h[b,j] * dX[b,c] * W2[j,h,c]  + b2-term

is computed as 64 accumulating fp8 DoubleRow matmuls

    k_psum += (h .* dX[:,c])^T  @  W2A[:, c]        (c = 0..63)

so the full F = h@W2 tensor (128x16384 f32 per stage) never exists and
never has to be evacuated from PSUM — the old kernel spent most of its
time on exactly that (ACT copy + DVE multiply + DVE segmented reduce).
The scaled activations h_c = fp8(h * dX[:,c]) are built on DVE+Pool from
a partition-replicated dX table streamed from DRAM (host-precomputed).
fp8 e4m3 DoubleRow gives 2x PE throughput; W2 is pre-scaled by 64 and
the 1/64 is folded into the RK4 combine scalars.
"""

import sys
import numpy as np

for _p in ("/opt/trn_rl_repo",):
    if _p not in sys.path:
        sys.path.insert(0, _p)

import ml_dtypes
from contextlib import ExitStack

import concourse.bass as bass
import concourse.bacc as bacc
import concourse.mybir as mybir
import concourse.tile as tile
from concourse.masks import make_identity
from concourse.bass_utils import run_bass_kernel_spmd

B, T, C, H = 1024, 11, 64, 256
NCORES = 8
BS = B // NCORES          # 128
HC = H * C                # 16384
NS = 21                   # distinct dX vectors: 10 knots + 10 mids + end

f32 = np.float32
bf16 = ml_dtypes.bfloat16
FP32 = mybir.dt.float32
BF16 = mybir.dt.bfloat16
FP8 = mybir.dt.float8e4
fp8np = mybir.dt.np(FP8)
AO = mybir.AluOpType
AF = mybir.ActivationFunctionType
DR = mybir.MatmulPerfMode.DoubleRow

W2SCALE = 64.0

# per-eval precision: 'b16' (safe) or 'f8' (2x PE, ~1e-2 extra error).
MODES = ['b16'] * 40

# quad ownership: which engine builds h_c quad q (1..15). DVE is ~2x Pool
# rate on bf16, so it takes ~2/3 of them.
QUAD_ENG = {q: ("pool" if q % 3 == 0 else "dve") for q in range(1, 16)}


def _stage_consts(t_span: np.ndarray):
    """Host-side f32 scalar constants mimicking the reference's fp32 ops."""
    t = np.asarray(t_span, dtype=f32)
    cs = []
    for i in range(T - 1):
        t0 = t[i]
        dt = f32(t[i + 1] - t0)
        tm = f32(t0 + f32(f32(0.5) * dt))
        idx_m = int(np.clip(np.searchsorted(t, tm, side="right") - 1, 0, T - 2))
        fm = f32(tm - t[idx_m])
        cs.append((float(dt), idx_m, float(fm)))
    fr_last = f32(t[T - 1] - t[T - 2])
    return cs, float(fr_last)


def _s_index(i, j):
    """dX vector index for eval j (0..3 = k1..k4) of step i."""
    if j == 0:
        return i
    if j in (1, 2):
        return 10 + i
    return i + 1 if i < T - 2 else 20


def _build_program(t_span: np.ndarray):
    cs, _ = _stage_consts(t_span)

    nc = bacc.Bacc("TRN2", target_bir_lowering=False, debug=False,
                   enable_asserts=False, num_devices=NCORES)

    a0t_d = nc.dram_tensor("a0t", [C, BS], BF16, kind="ExternalInput").ap()
    w1_d = nc.dram_tensor("w1", [128, 2 * H], BF16, kind="ExternalInput").ap()
    use_f8 = any(m == 'f8' for m in MODES)
    use_b16 = any(m == 'b16' for m in MODES)
    w2a_d = (nc.dram_tensor("w2a", [128, C * 2 * H], FP8, kind="ExternalInput").ap()
             if use_f8 else None)
    w2b_d = (nc.dram_tensor("w2b", [128, C * 2 * H], BF16, kind="ExternalInput").ap()
             if use_b16 else None)
    b1_d = nc.dram_tensor("b1", [128, 2], FP32, kind="ExternalInput").ap()
    b2rt_d = nc.dram_tensor("b2rt", [C, H], BF16, kind="ExternalInput").ap()
    winit_d = nc.dram_tensor("winit", [C, H], BF16, kind="ExternalInput").ap()
    wout_d = nc.dram_tensor("wout", [128, 2 * C], FP32, kind="ExternalInput").ap()
    binit_d = nc.dram_tensor("binit", [1, H], FP32, kind="ExternalInput").ap()
    bout_d = nc.dram_tensor("bout", [1, C], FP32, kind="ExternalInput").ap()
    dxt_d = nc.dram_tensor("dxt", [C, NS * BS], BF16, kind="ExternalInput").ap()
    dxbt_d = nc.dram_tensor("dxbt", [NS, 128, C * BS], BF16, kind="ExternalInput").ap()
    out_d = nc.dram_tensor("out", [BS, T * C], FP32, kind="ExternalOutput").ap()

    with tile.TileContext(nc) as tc, ExitStack() as ctx:
        const = ctx.enter_context(tc.tile_pool(name="const", bufs=1))
        dxpool = ctx.enter_context(tc.tile_pool(name="dxp", bufs=5))
        zpool = ctx.enter_context(tc.tile_pool(name="z", bufs=4))
        kbpool = ctx.enter_context(tc.tile_pool(name="kb", bufs=4))
        spool = ctx.enter_context(tc.tile_pool(name="sp", bufs=3))
        hcpool = ctx.enter_context(tc.tile_pool(name="hc", bufs=6))
        kpool = ctx.enter_context(tc.tile_pool(name="kps", bufs=3, space="PSUM"))
        tpool = ctx.enter_context(tc.tile_pool(name="tps", bufs=2, space="PSUM"))
        mpool = ctx.enter_context(tc.tile_pool(name="mps", bufs=3, space="PSUM"))

        # ---- resident tensors -------------------------------------------
        a0t_sb = const.tile([C, BS], BF16, tag="a0t")
        w1_sb = const.tile([128, 2 * H], BF16, tag="w1")
        w2a_sb = None
        w2b_sb = None
        if use_f8:
            w2a_sb = const.tile([128, C * 2 * H], FP8, tag="w2a")
        if use_b16:
            w2b_sb = const.tile([128, C * 2 * H], BF16, tag="w2b")
        b1_sb = const.tile([128, 2], FP32, tag="b1")
        b2rt_sb = const.tile([C, H], BF16, tag="b2rt")
        winit_sb = const.tile([C, H], BF16, tag="winit")
        wout_sb = const.tile([128, 2 * C], FP32, tag="wout")
        binit_sb = const.tile([1, H], FP32, tag="binit")
        bout_sb = const.tile([1, C], FP32, tag="bout")
        dxt_sb = const.tile([C, NS * BS], BF16, tag="dxt")
        ones1_sb = const.tile([1, 128], FP32, tag="ones1")
        ident = const.tile([128, 128], FP32, tag="ident")
        binit_rep = const.tile([128, H], FP32, tag="binit_rep")
        bout_rep = const.tile([128, C], FP32, tag="bout_rep")
        out_sb = const.tile([BS, T * C], FP32, tag="out_sb")

        nc.sync.dma_start(out=a0t_sb[:], in_=a0t_d)
        nc.sync.dma_start(out=w1_sb[:], in_=w1_d)
        if use_f8:
            nc.sync.dma_start(out=w2a_sb[:], in_=w2a_d)
        if use_b16:
            nc.sync.dma_start(out=w2b_sb[:], in_=w2b_d)
        nc.sync.dma_start(out=b1_sb[:], in_=b1_d)
        nc.sync.dma_start(out=b2rt_sb[:], in_=b2rt_d)
        nc.sync.dma_start(out=winit_sb[:], in_=winit_d)
        nc.sync.dma_start(out=wout_sb[:], in_=wout_d)
        nc.sync.dma_start(out=binit_sb[:], in_=binit_d)
        nc.sync.dma_start(out=bout_sb[:], in_=bout_d)
        nc.sync.dma_start(out=dxt_sb[:], in_=dxt_d)

        nc.vector.memset(ones1_sb[:], 1.0)
        make_identity(nc, ident[:])

        w2v = w2a_sb.rearrange("p (c k h) -> p c k h", c=C, k=2) if use_f8 else None
        w2bv = w2b_sb.rearrange("p (c k h) -> p c k h", c=C, k=2) if use_b16 else None

        # ---- dxbT streaming ---------------------------------------------
        dx_cache = {}

        def load_dxbT(s):
            if s in dx_cache:
                return
            t = dxpool.tile([128, C * BS], BF16, tag="dx")
            nc.sync.dma_start(out=t[:], in_=dxbt_d[s])
            dx_cache[s] = t

        for s0 in (0, 10, 1):
            load_dxbT(s0)

        # ---- replicated biases ------------------------------------------
        ps = mpool.tile([128, H], FP32, tag="m")
        nc.tensor.matmul(ps[:, 0:H], lhsT=ones1_sb[:], rhs=binit_sb[:],
                         start=True, stop=True)
        nc.scalar.copy(binit_rep[:], ps[:, 0:H])
        ps = mpool.tile([128, H], FP32, tag="m")
        nc.tensor.matmul(ps[:, 0:C], lhsT=ones1_sb[:], rhs=bout_sb[:],
                         start=True, stop=True)
        nc.scalar.copy(bout_rep[:], ps[:, 0:C])

        # ---- z0 ----------------------------------------------------------
        ps = mpool.tile([128, H], FP32, tag="m")
        nc.tensor.matmul(ps[:], lhsT=a0t_sb[:], rhs=winit_sb[:], start=True, stop=True)
        z = zpool.tile([BS, H], FP32, tag="z")
        nc.vector.tensor_tensor(out=z[:], in0=ps[:], in1=binit_rep[:], op=AO.add)

        # ---- one vector-field eval --------------------------------------
        def geval(zin, zbase, s, coef, want_zs, mode, emit_out_t=None):
            """k-eval at spline index s.  Returns (kb, zs):
            kb = coef*(k + bc)  [for the RK4 combine],
            zs = zin + coef*(k + bc) if want_zs else None."""
            dxbT = dx_cache[s]

            # transpose zin
            ztA = tpool.tile([128, 128], FP32, tag="t")
            ztB = tpool.tile([128, 128], FP32, tag="t")
            nc.tensor.transpose(ztA[:], zin[:, 0:128], ident[:])
            nc.tensor.transpose(ztB[:], zin[:, 128:256], ident[:])
            zTb = spool.tile([128, H], BF16, tag="zTb")
            nc.scalar.copy(zTb[:, 0:128], ztA[:])
            nc.scalar.copy(zTb[:, 128:256], ztB[:])

            # b2-term: bc = dX @ b2r.T ; bcs = coef*bc
            bc_ps = mpool.tile([128, H], FP32, tag="m")
            nc.tensor.matmul(bc_ps[:], lhsT=dxt_sb[:, s * BS:(s + 1) * BS],
                             rhs=b2rt_sb[:], start=True, stop=True)
            bcs = kbpool.tile([BS, H], FP32, tag="bcs")
            nc.scalar.activation(bcs[:], bc_ps[:], AF.Copy, scale=float(coef))
            zb = None
            if want_zs:
                zb = zpool.tile([BS, H], FP32, tag="zb")
                nc.vector.tensor_tensor(out=zb[:], in0=zbase[:], in1=bcs[:], op=AO.add)

            if emit_out_t is not None:
                zTf = spool.tile([128, H], FP32, tag="zTf")
                nc.scalar.copy(zTf[:, 0:128], ztA[:])
                nc.scalar.copy(zTf[:, 128:256], ztB[:])
                ot_ps = mpool.tile([128, H], FP32, tag="m")
                for kc in range(2):
                    nc.tensor.matmul(ot_ps[:, 0:C], lhsT=zTf[:, kc * 128:(kc + 1) * 128],
                                     rhs=wout_sb[:, kc * C:(kc + 1) * C],
                                     start=(kc == 0), stop=(kc == 1),
                                     skip_group_check=True)
                nc.vector.tensor_tensor(out=out_sb[:, emit_out_t * C:(emit_out_t + 1) * C],
                                        in0=ot_ps[:, 0:C], in1=bout_rep[:], op=AO.add)

            # hT = tanh(W1.T zT + b1), written to the r0 slots of ht4
            ht_ps = mpool.tile([128, H], FP32, tag="m")
            for hck in range(2):
                for kc in range(2):
                    nc.tensor.matmul(
                        ht_ps[:, hck * 128:(hck + 1) * 128],
                        lhsT=w1_sb[:, kc * H + hck * 128: kc * H + (hck + 1) * 128],
                        rhs=zTb[:, kc * 128:(kc + 1) * 128],
                        start=(kc == 0), stop=(kc == 1), skip_group_check=True)
            # ht4: [p, (kc, rep4, b)] (rep4 so quad builds get a dense in0)
            ht4 = spool.tile([128, 2 * 4 * 128], BF16, tag="ht4")
            for kc in range(2):
                nc.scalar.activation(ht4[:, kc * 512:kc * 512 + 128],
                                     ht_ps[:, kc * 128:(kc + 1) * 128],
                                     AF.Tanh, bias=b1_sb[:, kc:kc + 1], scale=1.0)
            # replicate r0 -> r1..r3 (ACT/DVE/Pool share)
            for kc in range(2):
                base = kc * 512
                nc.scalar.copy(ht4[:, base + 128:base + 256], ht4[:, base:base + 128])
                nc.vector.tensor_copy(out=ht4[:, base + 256:base + 384],
                                      in_=ht4[:, base:base + 128])
                nc.gpsimd.tensor_copy(out=ht4[:, base + 384:base + 512],
                                      in_=ht4[:, base:base + 128])
            dxv = dxbT.rearrange("p (c b) -> p c b", c=C)
            hdt_ = FP8 if mode == 'f8' else BF16

            kps = kpool.tile([128, H], FP32, tag="k")

            def build(q, eng, singles):
                """h_c for quad q in dense layout [p, (kc, c4, b)]."""
                hcq = hcpool.tile([128, 4 * 256], hdt_, tag="hc")
                if singles:
                    # narrow builds off the tanh output directly, so the
                    # first matmuls don't wait for the ht4 replication
                    for kc in range(2):
                        for ci in range(4):
                            e = nc.vector if (kc + ci) % 2 == 0 else nc.gpsimd
                            e.tensor_tensor(
                                out=hcq[:, kc * 512 + ci * 128: kc * 512 + (ci + 1) * 128],
                                in0=ht4[:, kc * 512:kc * 512 + 128],
                                in1=dxv[:, 4 * q + ci], op=AO.mult)
                else:
                    for kc in range(2):
                        eng.tensor_tensor(out=hcq[:, kc * 512:(kc + 1) * 512],
                                          in0=ht4[:, kc * 512:kc * 512 + 512],
                                          in1=dxv[:, 4 * q:4 * (q + 1)],
                                          op=AO.mult)
                return hcq

            def mm(q, hcq):
                hv = hcq.rearrange("p (k c b) -> p c k b", k=2, c=4)
                for ci in range(4):
                    c = 4 * q + ci
                    if mode == 'f8':
                        nc.tensor.matmul(kps[:], lhsT=hv[:, ci], rhs=w2v[:, c],
                                         start=(c == 0), stop=(c == C - 1),
                                         perf_mode=DR, skip_group_check=True)
                    else:
                        for kc in range(2):
                            nc.tensor.matmul(
                                kps[:], lhsT=hcq[:, kc * 512 + ci * 128: kc * 512 + (ci + 1) * 128],
                                rhs=w2bv[:, c, kc],
                                start=(c == 0 and kc == 0), stop=(c == C - 1 and kc == 1),
                                skip_group_check=True)

            mm(0, build(0, None, True))
            for q in range(1, 16):
                eng = nc.vector if QUAD_ENG[q] == "dve" else nc.gpsimd
                mm(q, build(q, eng, False))

            # kb = coef*(k/s + bc) ; zs = zb + coef*k/s
            sc = float(coef / W2SCALE) if mode == 'f8' else float(coef)
            kb = kbpool.tile([BS, H], FP32, tag="kb")
            zs = None
            if want_zs:
                zs = zpool.tile([BS, H], FP32, tag="zs")
                nc.vector.scalar_tensor_tensor(out=zs[:], in0=kps[:], scalar=sc,
                                               in1=zb[:], op0=AO.mult, op1=AO.add)
            nc.vector.scalar_tensor_tensor(out=kb[:], in0=kps[:], scalar=sc,
                                           in1=bcs[:], op0=AO.mult, op1=AO.add)
            return kb, zs

        # ---- RK4 time loop ----------------------------------------------
        for i in range(T - 1):
            dt_i, _, _ = cs[i]
            hdt = float(f32(f32(0.5) * f32(dt_i)))
            dt6 = float(f32(f32(dt_i) / f32(6.0)))

            kb1, z2 = geval(z, z, _s_index(i, 0), hdt, True, MODES[4 * i + 0], emit_out_t=i)
            # prefetch next step's dX tiles (emitted only now: a pool slot may
            # recycle a tile whose readers must all be emitted first)
            if i + 1 < T - 1:
                load_dxbT(10 + (i + 1))
                load_dxbT(_s_index(i + 1, 3))
            kb2, z3 = geval(z2, z, _s_index(i, 1), hdt, True, MODES[4 * i + 1])
            acc1 = kbpool.tile([BS, H], FP32, tag="acc")
            nc.vector.scalar_tensor_tensor(out=acc1[:], in0=kb2[:], scalar=2.0,
                                           in1=kb1[:], op0=AO.mult, op1=AO.add)
            kb3, z4 = geval(z3, z, _s_index(i, 2), float(dt_i), True, MODES[4 * i + 2])
            acc2 = kbpool.tile([BS, H], FP32, tag="acc")
            nc.gpsimd.tensor_tensor(out=acc2[:], in0=acc1[:], in1=kb3[:], op=AO.add)
            kb4, _ = geval(z4, z, _s_index(i, 3), dt6, False, MODES[4 * i + 3])
            acc3 = kbpool.tile([BS, H], FP32, tag="acc")
            nc.vector.scalar_tensor_tensor(out=acc3[:], in0=kb4[:], scalar=3.0,
                                           in1=acc2[:], op0=AO.mult, op1=AO.add)
            znew = zpool.tile([BS, H], FP32, tag="z")
            nc.vector.scalar_tensor_tensor(out=znew[:], in0=acc3[:],
                                           scalar=float(1.0 / 3.0),
                                           in1=z[:], op0=AO.mult, op1=AO.add)
            z = znew

        # ---- final out row (t = T-1) ------------------------------------
        ztA = tpool.tile([128, 128], FP32, tag="t")
        ztB = tpool.tile([128, 128], FP32, tag="t")
        nc.tensor.transpose(ztA[:], z[:, 0:128], ident[:])
        nc.tensor.transpose(ztB[:], z[:, 128:256], ident[:])
        zTf = spool.tile([128, H], FP32, tag="zTf")
        nc.scalar.copy(zTf[:, 0:128], ztA[:])
        nc.scalar.copy(zTf[:, 128:256], ztB[:])
        ot_ps = mpool.tile([128, H], FP32, tag="m")
        for kc in range(2):
            nc.tensor.matmul(ot_ps[:, 0:C], lhsT=zTf[:, kc * 128:(kc + 1) * 128],
                             rhs=wout_sb[:, kc * C:(kc + 1) * C],
                             start=(kc == 0), stop=(kc == 1))
        nc.vector.tensor_tensor(out=out_sb[:, (T - 1) * C:T * C],
                                in0=ot_ps[:, 0:C], in1=bout_rep[:], op=AO.add)

        nc.sync.dma_start(out=out_d, in_=out_sb[:])

    nc.compile()
    return nc


_CACHE = {}


def _get_program(t_span: np.ndarray):
    key = np.asarray(t_span, dtype=f32).tobytes()
    if key not in _CACHE:
        _CACHE[key] = _build_program(t_span)
    return _CACHE[key]


def _host_dx(coeffs: np.ndarray, t_span: np.ndarray):
    """All 21 distinct spline-derivative vectors, f32, shape [21, B, C]."""
    cs, fr_last = _stage_consts(t_span)
    a, b, two_c, three_d = np.split(np.asarray(coeffs, f32), 4, axis=-1)
    dx = np.empty((NS, B, C), dtype=f32)
    for s in range(10):
        dx[s] = b[:, s]
    for i in range(T - 1):
        _, im, fm = cs[i]
        fm = f32(fm)
        tmp = (three_d[:, im] * fm + two_c[:, im]).astype(f32)
        dx[10 + i] = (tmp * fm + b[:, im]).astype(f32)
    fr = f32(fr_last)
    tmp = (three_d[:, T - 2] * fr + two_c[:, T - 2]).astype(f32)
    dx[20] = (tmp * fr + b[:, T - 2]).astype(f32)
    return dx


def _make_in_maps(inputs):
    coeffs = np.ascontiguousarray(inputs["coeffs"], dtype=f32)
    assert coeffs.shape == (B, T - 1, 4 * C)
    t_span = np.asarray(inputs["t_span"], dtype=f32)

    W1 = np.asarray(inputs["W1"], f32)
    W2 = np.asarray(inputs["W2"], f32)
    W_out = np.asarray(inputs["W_out"], f32)

    shared = {
        "w1": np.ascontiguousarray(W1.reshape(2, 128, H).transpose(1, 0, 2)
                                   .reshape(128, 2 * H)).astype(bf16),
        "b1": np.ascontiguousarray(np.asarray(inputs["b1"], f32).reshape(2, 128).T),
        "b2rt": np.ascontiguousarray(
            np.asarray(inputs["b2"], f32).reshape(H, C).T).astype(bf16),
        "winit": np.ascontiguousarray(inputs["W_init"], dtype=f32).astype(bf16),
        "wout": np.ascontiguousarray(W_out.reshape(2, 128, C).transpose(1, 0, 2)
                                     .reshape(128, 2 * C)),
        "binit": np.ascontiguousarray(inputs["b_init"], dtype=f32).reshape(1, H),
        "bout": np.ascontiguousarray(inputs["b_out"], dtype=f32).reshape(1, C),
    }

    if any(m == 'f8' for m in MODES):
        w2s = (W2 * W2SCALE).reshape(2, 128, H, C)
        shared["w2a"] = np.ascontiguousarray(
            w2s.transpose(1, 3, 0, 2).reshape(128, C * 2 * H)).astype(fp8np)
    if any(m == 'b16' for m in MODES):
        w2s = W2.reshape(2, 128, H, C)
        shared["w2b"] = np.ascontiguousarray(
            w2s.transpose(1, 3, 0, 2).reshape(128, C * 2 * H)).astype(bf16)

    dx = _host_dx(coeffs, t_span)          # [21, B, C] f32
    dxb = dx.astype(bf16)

    in_maps = []
    for cid in range(NCORES):
        m = dict(shared)
        sl = slice(cid * BS, (cid + 1) * BS)
        m["a0t"] = np.ascontiguousarray(coeffs[sl, 0, 0:C].T).astype(bf16)
        # dxt[c, s*BS+b] = dX_s[b, c]
        m["dxt"] = np.ascontiguousarray(
            dxb[:, sl].transpose(2, 0, 1).reshape(C, NS * BS))
        # dxbt[s, p, c*BS+b] = dX_s[b, c]  (identical rows)
        rows = np.ascontiguousarray(
            dxb[:, sl].transpose(0, 2, 1).reshape(NS, 1, C * BS))
        m["dxbt"] = np.ascontiguousarray(np.broadcast_to(rows, (NS, 128, C * BS)))
        in_maps.append(m)
    return in_maps


def kernel(coeffs, t_span, W_init, b_init, W1, b1, W2, b2, W_out, b_out):
    nc = _get_program(t_span)
    in_maps = _make_in_maps(dict(coeffs=coeffs, t_span=t_span, W_init=W_init,
                                 b_init=b_init, W1=W1, b1=b1, W2=W2, b2=b2,
                                 W_out=W_out, b_out=b_out))
    res = run_bass_kernel_spmd(nc, in_maps, list(range(NCORES)))
    shards = [res.results[c]["out"].reshape(BS, T, C) for c in range(NCORES)]
    return np.ascontiguousarray(np.concatenate(shards, axis=0), dtype=f32)


if __name__ == "__main__":
    rng = np.random.default_rng(0)
    demo = dict(
        coeffs=(rng.standard_normal((B, T - 1, 4 * C)) * 0.5).astype(f32),
        t_span=(np.arange(T) * 0.05).astype(f32),
        W_init=(rng.standard_normal((C, H)) / 8).astype(f32),
        b_init=(rng.standard_normal((H,)) * 0.01).astype(f32),
        W1=(rng.standard_normal((H, H)) / 16).astype(f32),
        b1=(rng.standard_normal((H,)) * 0.01).astype(f32),
        W2=(rng.standard_normal((H, HC)) / 16).astype(f32),
        b2=(rng.standard_normal((HC,)) * 0.01).astype(f32),
        W_out=(rng.standard_normal((H, C)) / 16).astype(f32),
        b_out=np.zeros((C,), f32),
    )
    out = kernel(**demo)
    print("out", out.shape, out.dtype, float(np.abs(out).max()))


# revision 11
# speedup vs baseline: 1.4104x; 1.1857x over previous
"""Neural CDE (RK4, 10 steps) Trainium2 Bass/Tile kernel — fp8 rewrite.

Data-parallel over batch: B=1024 split as 128 per core across 8 NeuronCores.
Weights replicated; no collectives.

Key idea vs the bf16 baseline: fold the spline derivative dX into the
matmul's *stationary* operand.  For each channel c the per-stage vector
field contraction

    k[b,h] = sum_{j,c} h[b,j] * dX[b,c] * W2[j,h,c]  + b2-term

is computed as 64 accumulating fp8 DoubleRow matmuls

    k_psum += (h .* dX[:,c])^T  @  W2A[:, c]        (c = 0..63)

so the full F = h@W2 tensor (128x16384 f32 per stage) never exists and
never has to be evacuated from PSUM — the old kernel spent most of its
time on exactly that (ACT copy + DVE multiply + DVE segmented reduce).
The scaled activations h_c = fp8(h * dX[:,c]) are built on DVE+Pool from
a partition-replicated dX table streamed from DRAM (host-precomputed).
fp8 e4m3 DoubleRow gives 2x PE throughput; W2 is pre-scaled by 64 and
the 1/64 is folded into the RK4 combine scalars.
"""

import sys
import numpy as np

for _p in ("/opt/trn_rl_repo",):
    if _p not in sys.path:
        sys.path.insert(0, _p)

import ml_dtypes
from contextlib import ExitStack

import concourse.bass as bass
import concourse.bacc as bacc
import concourse.mybir as mybir
import concourse.tile as tile
from concourse.masks import make_identity
from concourse.bass_utils import run_bass_kernel_spmd

B, T, C, H = 1024, 11, 64, 256
NCORES = 8
BS = B // NCORES          # 128
HC = H * C                # 16384
NS = 21                   # distinct dX vectors: 10 knots + 10 mids + end

f32 = np.float32
bf16 = ml_dtypes.bfloat16
FP32 = mybir.dt.float32
BF16 = mybir.dt.bfloat16
FP8 = mybir.dt.float8e4
fp8np = mybir.dt.np(FP8)
AO = mybir.AluOpType
AF = mybir.ActivationFunctionType
DR = mybir.MatmulPerfMode.DoubleRow

W2SCALE = 64.0

# per-eval precision: 'b16' (safe) or 'f8' (2x PE, ~1e-2 extra error).
MODES = ['b16'] * 40

# quad ownership: which engine builds h_c quad q (1..15). DVE is ~2x Pool
# rate on bf16, so it takes ~2/3 of them.
QUAD_ENG = {q: ("dve" if q % 2 == 1 else "pool") for q in range(1, 16)}


def _stage_consts(t_span: np.ndarray):
    """Host-side f32 scalar constants mimicking the reference's fp32 ops."""
    t = np.asarray(t_span, dtype=f32)
    cs = []
    for i in range(T - 1):
        t0 = t[i]
        dt = f32(t[i + 1] - t0)
        tm = f32(t0 + f32(f32(0.5) * dt))
        idx_m = int(np.clip(np.searchsorted(t, tm, side="right") - 1, 0, T - 2))
        fm = f32(tm - t[idx_m])
        cs.append((float(dt), idx_m, float(fm)))
    fr_last = f32(t[T - 1] - t[T - 2])
    return cs, float(fr_last)


def _s_index(i, j):
    """dX vector index for eval j (0..3 = k1..k4) of step i."""
    if j == 0:
        return i
    if j in (1, 2):
        return 10 + i
    return i + 1 if i < T - 2 else 20


def _build_program(t_span: np.ndarray):
    cs, _ = _stage_consts(t_span)

    nc = bacc.Bacc("TRN2", target_bir_lowering=False, debug=False,
                   enable_asserts=False, num_devices=NCORES)

    a0t_d = nc.dram_tensor("a0t", [C, BS], BF16, kind="ExternalInput").ap()
    w1_d = nc.dram_tensor("w1", [128, 2 * H], BF16, kind="ExternalInput").ap()
    use_f8 = any(m == 'f8' for m in MODES)
    use_b16 = any(m == 'b16' for m in MODES)
    w2a_d = (nc.dram_tensor("w2a", [128, C * 2 * H], FP8, kind="ExternalInput").ap()
             if use_f8 else None)
    w2b_d = (nc.dram_tensor("w2b", [128, C * 2 * H], BF16, kind="ExternalInput").ap()
             if use_b16 else None)
    b1_d = nc.dram_tensor("b1", [128, 2], FP32, kind="ExternalInput").ap()
    b2rt_d = nc.dram_tensor("b2rt", [C, H], BF16, kind="ExternalInput").ap()
    winit_d = nc.dram_tensor("winit", [C, H], BF16, kind="ExternalInput").ap()
    wout_d = nc.dram_tensor("wout", [128, 2 * C], FP32, kind="ExternalInput").ap()
    binit_d = nc.dram_tensor("binit", [1, H], FP32, kind="ExternalInput").ap()
    bout_d = nc.dram_tensor("bout", [1, C], FP32, kind="ExternalInput").ap()
    dxt_d = nc.dram_tensor("dxt", [C, NS * BS], BF16, kind="ExternalInput").ap()
    dxbt_d = nc.dram_tensor("dxbt", [NS, 128, C * BS], BF16, kind="ExternalInput").ap()
    out_d = nc.dram_tensor("out", [BS, T * C], FP32, kind="ExternalOutput").ap()

    with tile.TileContext(nc) as tc, ExitStack() as ctx:
        const = ctx.enter_context(tc.tile_pool(name="const", bufs=1))
        dxpool = ctx.enter_context(tc.tile_pool(name="dxp", bufs=5))
        zpool = ctx.enter_context(tc.tile_pool(name="z", bufs=3))
        kbpool = ctx.enter_context(tc.tile_pool(name="kb", bufs=4))
        auxpool = ctx.enter_context(tc.tile_pool(name="aux", bufs=2))
        spool = ctx.enter_context(tc.tile_pool(name="sp", bufs=3))
        hcpool = ctx.enter_context(tc.tile_pool(name="hc", bufs=6))
        kpool = ctx.enter_context(tc.tile_pool(name="kps", bufs=3, space="PSUM"))
        tpool = ctx.enter_context(tc.tile_pool(name="tps", bufs=2, space="PSUM"))
        mpool = ctx.enter_context(tc.tile_pool(name="mps", bufs=3, space="PSUM"))

        # ---- resident tensors -------------------------------------------
        a0t_sb = const.tile([C, BS], BF16, tag="a0t")
        w1_sb = const.tile([128, 2 * H], BF16, tag="w1")
        w2a_sb = None
        w2b_sb = None
        if use_f8:
            w2a_sb = const.tile([128, C * 2 * H], FP8, tag="w2a")
        if use_b16:
            w2b_sb = const.tile([128, C * 2 * H], BF16, tag="w2b")
        b1_sb = const.tile([128, 2], FP32, tag="b1")
        b2rt_sb = const.tile([C, H], BF16, tag="b2rt")
        winit_sb = const.tile([C, H], BF16, tag="winit")
        wout_sb = const.tile([128, 2 * C], FP32, tag="wout")
        binit_sb = const.tile([1, H], FP32, tag="binit")
        bout_sb = const.tile([1, C], FP32, tag="bout")
        dxt_sb = const.tile([C, NS * BS], BF16, tag="dxt")
        ones1_sb = const.tile([1, 128], FP32, tag="ones1")
        ident = const.tile([128, 128], FP32, tag="ident")
        binit_rep = const.tile([128, H], FP32, tag="binit_rep")
        bout_rep = const.tile([128, C], FP32, tag="bout_rep")
        out_sb = const.tile([BS, T * C], FP32, tag="out_sb")

        nc.sync.dma_start(out=a0t_sb[:], in_=a0t_d)
        nc.sync.dma_start(out=w1_sb[:], in_=w1_d)
        if use_f8:
            nc.sync.dma_start(out=w2a_sb[:], in_=w2a_d)
        if use_b16:
            nc.sync.dma_start(out=w2b_sb[:], in_=w2b_d)
        nc.sync.dma_start(out=b1_sb[:], in_=b1_d)
        nc.sync.dma_start(out=b2rt_sb[:], in_=b2rt_d)
        nc.sync.dma_start(out=winit_sb[:], in_=winit_d)
        nc.sync.dma_start(out=wout_sb[:], in_=wout_d)
        nc.sync.dma_start(out=binit_sb[:], in_=binit_d)
        nc.sync.dma_start(out=bout_sb[:], in_=bout_d)
        nc.sync.dma_start(out=dxt_sb[:], in_=dxt_d)

        nc.vector.memset(ones1_sb[:], 1.0)
        make_identity(nc, ident[:])

        w2v = w2a_sb.rearrange("p (c k h) -> p c k h", c=C, k=2) if use_f8 else None
        w2bv = w2b_sb.rearrange("p (c k h) -> p c k h", c=C, k=2) if use_b16 else None

        # ---- dxbT streaming ---------------------------------------------
        dx_cache = {}

        def load_dxbT(s):
            if s in dx_cache:
                return
            t = dxpool.tile([128, C * BS], BF16, tag="dx")
            nc.sync.dma_start(out=t[:], in_=dxbt_d[s])
            dx_cache[s] = t

        for s0 in (0, 10, 1):
            load_dxbT(s0)

        # ---- replicated biases ------------------------------------------
        ps = mpool.tile([128, H], FP32, tag="m")
        nc.tensor.matmul(ps[:, 0:H], lhsT=ones1_sb[:], rhs=binit_sb[:],
                         start=True, stop=True)
        nc.scalar.copy(binit_rep[:], ps[:, 0:H])
        ps = mpool.tile([128, H], FP32, tag="m")
        nc.tensor.matmul(ps[:, 0:C], lhsT=ones1_sb[:], rhs=bout_sb[:],
                         start=True, stop=True)
        nc.scalar.copy(bout_rep[:], ps[:, 0:C])

        # ---- z0 ----------------------------------------------------------
        ps = mpool.tile([128, H], FP32, tag="m")
        nc.tensor.matmul(ps[:], lhsT=a0t_sb[:], rhs=winit_sb[:], start=True, stop=True)
        z = zpool.tile([BS, H], FP32, tag="z")
        nc.vector.tensor_tensor(out=z[:], in0=ps[:], in1=binit_rep[:], op=AO.add)

        # ---- one vector-field eval --------------------------------------
        def geval(zin, zbase, s, coef, want_zs, mode, emit_out_t=None):
            """k-eval at spline index s.  Returns (kb, zs):
            kb = coef*(k + bc)  [for the RK4 combine],
            zs = zin + coef*(k + bc) if want_zs else None."""
            dxbT = dx_cache[s]

            # transpose zin
            ztA = tpool.tile([128, 128], FP32, tag="t")
            ztB = tpool.tile([128, 128], FP32, tag="t")
            nc.tensor.transpose(ztA[:], zin[:, 0:128], ident[:])
            nc.tensor.transpose(ztB[:], zin[:, 128:256], ident[:])
            zTb = spool.tile([128, H], BF16, tag="zTb")
            nc.scalar.copy(zTb[:, 0:128], ztA[:])
            nc.scalar.copy(zTb[:, 128:256], ztB[:])

            # b2-term: bc = dX @ b2r.T ; bcs = coef*bc
            bc_ps = mpool.tile([128, H], FP32, tag="m")
            nc.tensor.matmul(bc_ps[:], lhsT=dxt_sb[:, s * BS:(s + 1) * BS],
                             rhs=b2rt_sb[:], start=True, stop=True)
            bcs = auxpool.tile([BS, H], FP32, tag="bcs")
            nc.scalar.activation(bcs[:], bc_ps[:], AF.Copy, scale=float(coef))
            zb = None
            if want_zs:
                zb = zpool.tile([BS, H], FP32, tag="zb")
                nc.vector.scalar_tensor_tensor(out=zb[:], in0=bc_ps[:], scalar=float(coef),
                                               in1=zbase[:], op0=AO.mult, op1=AO.add)

            if emit_out_t is not None:
                zTf = spool.tile([128, H], FP32, tag="zTf")
                nc.scalar.copy(zTf[:, 0:128], ztA[:])
                nc.scalar.copy(zTf[:, 128:256], ztB[:])
                ot_ps = mpool.tile([128, H], FP32, tag="m")
                for kc in range(2):
                    nc.tensor.matmul(ot_ps[:, 0:C], lhsT=zTf[:, kc * 128:(kc + 1) * 128],
                                     rhs=wout_sb[:, kc * C:(kc + 1) * C],
                                     start=(kc == 0), stop=(kc == 1),
                                     skip_group_check=True)
                nc.vector.tensor_tensor(out=out_sb[:, emit_out_t * C:(emit_out_t + 1) * C],
                                        in0=ot_ps[:, 0:C], in1=bout_rep[:], op=AO.add)

            # hT = tanh(W1.T zT + b1), written to the r0 slots of ht4
            ht_ps = mpool.tile([128, H], FP32, tag="m")
            for hck in range(2):
                for kc in range(2):
                    nc.tensor.matmul(
                        ht_ps[:, hck * 128:(hck + 1) * 128],
                        lhsT=w1_sb[:, kc * H + hck * 128: kc * H + (hck + 1) * 128],
                        rhs=zTb[:, kc * 128:(kc + 1) * 128],
                        start=(kc == 0), stop=(kc == 1), skip_group_check=True)
            # ht4: [p, (kc, rep4, b)] (rep4 so quad builds get a dense in0)
            ht4 = spool.tile([128, 2 * 4 * 128], BF16, tag="ht4")
            for kc in range(2):
                nc.scalar.activation(ht4[:, kc * 512:kc * 512 + 128],
                                     ht_ps[:, kc * 128:(kc + 1) * 128],
                                     AF.Tanh, bias=b1_sb[:, kc:kc + 1], scale=1.0)
            # replicate r0 -> r1..r3 (ACT; it is otherwise idle)
            for kc in range(2):
                base = kc * 512
                nc.scalar.copy(ht4[:, base + 128:base + 256], ht4[:, base:base + 128])
                nc.scalar.copy(ht4[:, base + 256:base + 512], ht4[:, base:base + 256])
            dxv = dxbT.rearrange("p (c b) -> p c b", c=C)
            hdt_ = FP8 if mode == 'f8' else BF16

            kpsA = kpool.tile([128, H], FP32, tag="k")
            kpsB = kpool.tile([128, H], FP32, tag="k")

            def build(q, eng, singles):
                """h_c for quad q in dense layout [p, (kc, c4, b)]."""
                hcq = hcpool.tile([128, 4 * 256], hdt_, tag="hc")
                if singles:
                    # narrow builds off the tanh output directly, so the
                    # first matmuls don't wait for the ht4 replication
                    for kc in range(2):
                        for ci in range(4):
                            e = nc.vector if (kc + ci) % 2 == 0 else nc.gpsimd
                            e.tensor_tensor(
                                out=hcq[:, kc * 512 + ci * 128: kc * 512 + (ci + 1) * 128],
                                in0=ht4[:, kc * 512:kc * 512 + 128],
                                in1=dxv[:, 4 * q + ci], op=AO.mult)
                else:
                    for kc in range(2):
                        eng.tensor_tensor(out=hcq[:, kc * 512:(kc + 1) * 512],
                                          in0=ht4[:, kc * 512:kc * 512 + 512],
                                          in1=dxv[:, 4 * q:4 * (q + 1)],
                                          op=AO.mult)
                return hcq

            def mm(q, hcq):
                hv = hcq.rearrange("p (k c b) -> p c k b", k=2, c=4)
                kps = kpsA if q < 8 else kpsB
                base = 0 if q < 8 else 32
                for ci in range(4):
                    c = 4 * q + ci
                    if mode == 'f8':
                        nc.tensor.matmul(kps[:], lhsT=hv[:, ci], rhs=w2v[:, c],
                                         start=(c == base), stop=(c == base + 31),
                                         perf_mode=DR, skip_group_check=True)
                    else:
                        for kc in range(2):
                            nc.tensor.matmul(
                                kps[:], lhsT=hcq[:, kc * 512 + ci * 128: kc * 512 + (ci + 1) * 128],
                                rhs=w2bv[:, c, kc],
                                start=(c == base and kc == 0), stop=(c == base + 31 and kc == 1),
                                skip_group_check=True)

            mm(0, build(0, None, True))
            for q in range(1, 16):
                eng = nc.vector if QUAD_ENG[q] == "dve" else nc.gpsimd
                mm(q, build(q, eng, False))

            # kb = coef*(k/s + bc) ; zs = zb + coef*k/s  (k = kpsA + kpsB)
            sc = float(coef / W2SCALE) if mode == 'f8' else float(coef)
            kb = kbpool.tile([BS, H], FP32, tag="kb")
            kbh = auxpool.tile([BS, H], FP32, tag="kbh")
            nc.vector.scalar_tensor_tensor(out=kbh[:], in0=kpsA[:], scalar=sc,
                                           in1=bcs[:], op0=AO.mult, op1=AO.add)
            zs = None
            if want_zs:
                zsh = zpool.tile([BS, H], FP32, tag="zsh")
                zs = zpool.tile([BS, H], FP32, tag="zs")
                nc.vector.scalar_tensor_tensor(out=zsh[:], in0=kpsA[:], scalar=sc,
                                               in1=zb[:], op0=AO.mult, op1=AO.add)
                nc.vector.scalar_tensor_tensor(out=zs[:], in0=kpsB[:], scalar=sc,
                                               in1=zsh[:], op0=AO.mult, op1=AO.add)
            nc.vector.scalar_tensor_tensor(out=kb[:], in0=kpsB[:], scalar=sc,
                                           in1=kbh[:], op0=AO.mult, op1=AO.add)
            return kb, zs

        # ---- RK4 time loop ----------------------------------------------
        for i in range(T - 1):
            dt_i, _, _ = cs[i]
            hdt = float(f32(f32(0.5) * f32(dt_i)))
            dt6 = float(f32(f32(dt_i) / f32(6.0)))

            kb1, z2 = geval(z, z, _s_index(i, 0), hdt, True, MODES[4 * i + 0], emit_out_t=i)
            # prefetch next step's dX tiles (emitted only now: a pool slot may
            # recycle a tile whose readers must all be emitted first)
            if i + 1 < T - 1:
                load_dxbT(10 + (i + 1))
                load_dxbT(_s_index(i + 1, 3))
            kb2, z3 = geval(z2, z, _s_index(i, 1), hdt, True, MODES[4 * i + 1])
            acc1 = auxpool.tile([BS, H], FP32, tag="acc")
            nc.vector.scalar_tensor_tensor(out=acc1[:], in0=kb2[:], scalar=2.0,
                                           in1=kb1[:], op0=AO.mult, op1=AO.add)
            kb3, z4 = geval(z3, z, _s_index(i, 2), float(dt_i), True, MODES[4 * i + 2])
            acc2 = auxpool.tile([BS, H], FP32, tag="acc")
            nc.gpsimd.tensor_tensor(out=acc2[:], in0=acc1[:], in1=kb3[:], op=AO.add)
            kb4, _ = geval(z4, z, _s_index(i, 3), dt6, False, MODES[4 * i + 3])
            acc3 = auxpool.tile([BS, H], FP32, tag="acc")
            nc.vector.scalar_tensor_tensor(out=acc3[:], in0=kb4[:], scalar=3.0,
                                           in1=acc2[:], op0=AO.mult, op1=AO.add)
            znew = zpool.tile([BS, H], FP32, tag="z")
            nc.vector.scalar_tensor_tensor(out=znew[:], in0=acc3[:],
                                           scalar=float(1.0 / 3.0),
                                           in1=z[:], op0=AO.mult, op1=AO.add)
            z = znew

        # ---- final out row (t = T-1) ------------------------------------
        ztA = tpool.tile([128, 128], FP32, tag="t")
        ztB = tpool.tile([128, 128], FP32, tag="t")
        nc.tensor.transpose(ztA[:], z[:, 0:128], ident[:])
        nc.tensor.transpose(ztB[:], z[:, 128:256], ident[:])
        zTf = spool.tile([128, H], FP32, tag="zTf")
        nc.scalar.copy(zTf[:, 0:128], ztA[:])
        nc.scalar.copy(zTf[:, 128:256], ztB[:])
        ot_ps = mpool.tile([128, H], FP32, tag="m")
        for kc in range(2):
            nc.tensor.matmul(ot_ps[:, 0:C], lhsT=zTf[:, kc * 128:(kc + 1) * 128],
                             rhs=wout_sb[:, kc * C:(kc + 1) * C],
                             start=(kc == 0), stop=(kc == 1))
        nc.vector.tensor_tensor(out=out_sb[:, (T - 1) * C:T * C],
                                in0=ot_ps[:, 0:C], in1=bout_rep[:], op=AO.add)

        nc.sync.dma_start(out=out_d, in_=out_sb[:])

    nc.compile()
    return nc


_CACHE = {}


def _get_program(t_span: np.ndarray):
    key = np.asarray(t_span, dtype=f32).tobytes()
    if key not in _CACHE:
        _CACHE[key] = _build_program(t_span)
    return _CACHE[key]


def _host_dx(coeffs: np.ndarray, t_span: np.ndarray):
    """All 21 distinct spline-derivative vectors, f32, shape [21, B, C]."""
    cs, fr_last = _stage_consts(t_span)
    a, b, two_c, three_d = np.split(np.asarray(coeffs, f32), 4, axis=-1)
    dx = np.empty((NS, B, C), dtype=f32)
    for s in range(10):
        dx[s] = b[:, s]
    for i in range(T - 1):
        _, im, fm = cs[i]
        fm = f32(fm)
        tmp = (three_d[:, im] * fm + two_c[:, im]).astype(f32)
        dx[10 + i] = (tmp * fm + b[:, im]).astype(f32)
    fr = f32(fr_last)
    tmp = (three_d[:, T - 2] * fr + two_c[:, T - 2]).astype(f32)
    dx[20] = (tmp * fr + b[:, T - 2]).astype(f32)
    return dx


def _make_in_maps(inputs):
    coeffs = np.ascontiguousarray(inputs["coeffs"], dtype=f32)
    assert coeffs.shape == (B, T - 1, 4 * C)
    t_span = np.asarray(inputs["t_span"], dtype=f32)

    W1 = np.asarray(inputs["W1"], f32)
    W2 = np.asarray(inputs["W2"], f32)
    W_out = np.asarray(inputs["W_out"], f32)

    shared = {
        "w1": np.ascontiguousarray(W1.reshape(2, 128, H).transpose(1, 0, 2)
                                   .reshape(128, 2 * H)).astype(bf16),
        "b1": np.ascontiguousarray(np.asarray(inputs["b1"], f32).reshape(2, 128).T),
        "b2rt": np.ascontiguousarray(
            np.asarray(inputs["b2"], f32).reshape(H, C).T).astype(bf16),
        "winit": np.ascontiguousarray(inputs["W_init"], dtype=f32).astype(bf16),
        "wout": np.ascontiguousarray(W_out.reshape(2, 128, C).transpose(1, 0, 2)
                                     .reshape(128, 2 * C)),
        "binit": np.ascontiguousarray(inputs["b_init"], dtype=f32).reshape(1, H),
        "bout": np.ascontiguousarray(inputs["b_out"], dtype=f32).reshape(1, C),
    }

    if any(m == 'f8' for m in MODES):
        w2s = (W2 * W2SCALE).reshape(2, 128, H, C)
        shared["w2a"] = np.ascontiguousarray(
            w2s.transpose(1, 3, 0, 2).reshape(128, C * 2 * H)).astype(fp8np)
    if any(m == 'b16' for m in MODES):
        w2s = W2.reshape(2, 128, H, C)
        shared["w2b"] = np.ascontiguousarray(
            w2s.transpose(1, 3, 0, 2).reshape(128, C * 2 * H)).astype(bf16)

    dx = _host_dx(coeffs, t_span)          # [21, B, C] f32
    dxb = dx.astype(bf16)

    in_maps = []
    for cid in range(NCORES):
        m = dict(shared)
        sl = slice(cid * BS, (cid + 1) * BS)
        m["a0t"] = np.ascontiguousarray(coeffs[sl, 0, 0:C].T).astype(bf16)
        # dxt[c, s*BS+b] = dX_s[b, c]
        m["dxt"] = np.ascontiguousarray(
            dxb[:, sl].transpose(2, 0, 1).reshape(C, NS * BS))
        # dxbt[s, p, c*BS+b] = dX_s[b, c]  (identical rows)
        rows = np.ascontiguousarray(
            dxb[:, sl].transpose(0, 2, 1).reshape(NS, 1, C * BS))
        m["dxbt"] = np.ascontiguousarray(np.broadcast_to(rows, (NS, 128, C * BS)))
        in_maps.append(m)
    return in_maps


def kernel(coeffs, t_span, W_init, b_init, W1, b1, W2, b2, W_out, b_out):
    nc = _get_program(t_span)
    in_maps = _make_in_maps(dict(coeffs=coeffs, t_span=t_span, W_init=W_init,
                                 b_init=b_init, W1=W1, b1=b1, W2=W2, b2=b2,
                                 W_out=W_out, b_out=b_out))
    res = run_bass_kernel_spmd(nc, in_maps, list(range(NCORES)))
    shards = [res.results[c]["out"].reshape(BS, T, C) for c in range(NCORES)]
    return np.ascontiguousarray(np.concatenate(shards, axis=0), dtype=f32)


if __name__ == "__main__":
    rng = np.random.default_rng(0)
    demo = dict(
        coeffs=(rng.standard_normal((B, T - 1, 4 * C)) * 0.5).astype(f32),
        t_span=(np.arange(T) * 0.05).astype(f32),
        W_init=(rng.standard_normal((C, H)) / 8).astype(f32),
        b_init=(rng.standard_normal((H,)) * 0.01).astype(f32),
        W1=(rng.standard_normal((H, H)) / 16).astype(f32),
        b1=(rng.standard_normal((H,)) * 0.01).astype(f32),
        W2=(rng.standard_normal((H, HC)) / 16).astype(f32),
        b2=(rng.standard_normal((HC,)) * 0.01).astype(f32),
        W_out=(rng.standard_normal((H, C)) / 16).astype(f32),
        b_out=np.zeros((C,), f32),
    )
    out = kernel(**demo)
    print("out", out.shape, out.dtype, float(np.abs(out).max()))
